# revision 2
# baseline (speedup 1.0000x reference)
"""Trainium2 Bass kernel v2 for nn_Attention_38491496907192.

LayerNorm -> QKV projection -> cosine-sim causal attention (8 heads) -> out
projection, for x [2, 2048, 1024], w_qkv [1024, 1536], w_out [512, 1024].

Sharding (8 NeuronCores): core i handles batch i//4 and head pair
(i%4)*2 .. +2. Each core emits a partial [2048, 1024] fp16 output; host
sums 4 partials per batch.

v2 redesign vs baseline (389us): LayerNorm is never materialized.
 - q/k: l2-normalization makes the per-token rsig scale a no-op, so
   q_eff = x@Wq' - mu*colsum(Wq') + sig*qb, computed entirely on the PE by
   appending two rank-1 outer-product matmuls (lhsT [1,128]) to the psum
   accumulation. mu/sig come from LN stats.
 - v: v = rsig*(x@Wv' - mu*cs_v + sig*vb); the rsig multiply folds into
   the psum->sbuf drain as a per-partition tensor_scalar.
 - LN stats in token-partition layout: squares split DVE/ACT, strip sums
   via free-size-1 PE matmuls, stats math on [128,16] tiles, PE-transpose
   + pack-DMA yields mu/sig as [1,2048] rows for the outer products.
 - 1/||q|| via exp(-0.5*ln(s)) on ACT: ln+exp+copy live in one activation
   table (natural_log_exp_and_others) so ACT never reloads tables.
 - Broadcasts via PE indicator matmuls (ind2 [2,128] lhsT), not gpsimd.
 - Causal masks precomputed host-side, applied as fp16 DVE multiplies.
 - Denominators: V carries interleaved ones-columns so PV yields den rows;
   reciprocal on DVE at partition base 64, row-DMA to base 0, fp32
   indicator-matmul broadcast, divide folded into the po drain.
 - Software pipelined: each megablock's denominator + out-projection tail
   is emitted during the next megablock's projection phase; QK runs one
   step ahead of PV so exp latency hides.
"""

import sys

sys.path.insert(0, "/opt/trn_rl_repo")

import numpy as np
from contextlib import ExitStack

import concourse.bass as bass
import concourse.tile as tile
from concourse import bacc, mybir
from concourse import bass_utils

_orig_get_tables = bacc.get_activation_tables


def _only_ln_exp_tables(arch):
    """Force every ACT table choice to natural_log_exp_and_others (which
    holds exp+ln+copy+square - the only functions this kernel uses) so the
    act-table-load pass emits exactly one load instead of ping-ponging
    between the exp-only and ln-only sets (~1.3us per reload)."""
    tabs = _orig_get_tables(arch)
    out = {}
    for name, s in tabs.items():
        out[name] = s if name == "natural_log_exp_and_others" else set()
    return out


bacc.get_activation_tables = _only_ln_exp_tables

AF = mybir.ActivationFunctionType
OP = mybir.AluOpType
F16 = mybir.dt.float16
F32 = mybir.dt.float32

B, N, DIM = 2, 2048, 1024
H, D = 8, 64
P = 128
NBLK = N // P            # 16 row blocks
KT = DIM // P            # 8 contraction strips
QM = N // 512            # 4 query megablocks
N_CORES = 8
LN_EPS = 1e-5
L2_EPS = 1e-12
SCALE = 8.0
INV_D = 1.0 / DIM

_CACHE = {}


def build_nc():
    nc = bacc.Bacc("TRN2", target_bir_lowering=False, debug=False,
                   num_devices=N_CORES)
    xT_d = nc.dram_tensor("xT", [DIM, N], F16, kind="ExternalInput").ap()
    wqk_d = nc.dram_tensor("wqk", [P, KT, 256], F16, kind="ExternalInput").ap()
    wv_d = nc.dram_tensor("wv", [P, KT, 130], F16, kind="ExternalInput").ap()
    wout_d = nc.dram_tensor("wout", [P, DIM], F16, kind="ExternalInput").ap()
    ncsqk_d = nc.dram_tensor("ncsqk", [1, 256], F16, kind="ExternalInput").ap()
    qkb_d = nc.dram_tensor("qkb", [1, 256], F16, kind="ExternalInput").ap()
    ncsv_d = nc.dram_tensor("ncsv", [1, 130], F16, kind="ExternalInput").ap()
    vb1_d = nc.dram_tensor("vb1", [1, 130], F16, kind="ExternalInput").ap()
    mask_d = nc.dram_tensor("maskc", [P, 2048], F16, kind="ExternalInput").ap()
    ind2_d = nc.dram_tensor("ind2", [2, P], F16, kind="ExternalInput").ap()
    ind2f_d = nc.dram_tensor("ind2f", [2, P], F32, kind="ExternalInput").ap()
    id128_d = nc.dram_tensor("id128", [P, P], F16, kind="ExternalInput").ap()
    y_d = nc.dram_tensor("y", [N, DIM], F16, kind="ExternalOutput").ap()

    with tile.TileContext(nc) as tc, ExitStack() as ctx:
        const = ctx.enter_context(tc.tile_pool(name="const", bufs=1))
        xts = [const.tile([P, N], F16, name=f"xts{kt}") for kt in range(KT)]
        wqk_sb = const.tile([P, KT, 256], F16, name="wqk")
        wv_sb = const.tile([P, KT, 130], F16, name="wv")
        wout_sb = const.tile([P, DIM], F16, name="wout")
        ncsqk = const.tile([1, 256], F16, name="ncsqk")
        qkb = const.tile([1, 256], F16, name="qkb")
        ncsv = const.tile([1, 130], F16, name="ncsv")
        vb1 = const.tile([1, 130], F16, name="vb1")
        maskc = const.tile([P, 2048], F16, name="maskc")
        ind2 = const.tile([2, P], F16, name="ind2")
        ind2f = const.tile([2, P], F32, name="ind2f")
        id128 = const.tile([P, P], F16, name="id128")
        ones1 = const.tile([P, 1], F16, name="ones1")
        epsA = const.tile([P, 1], F32, name="epsA")
        epsB = const.tile([P, 1], F32, name="epsB")
        ones2 = const.tile([P, 2], F16, name="ones2")
        qT = const.tile([P, N], F16, name="qT")
        kT = const.tile([P, N], F16, name="kT")
        V = const.tile([P, NBLK, 130], F16, name="V")
        mu1 = const.tile([1, N], F16, name="mu1")
        sig1 = const.tile([1, N], F16, name="sig1")
        rsigc = const.tile([P, NBLK], F32, name="rsigc")
        sig16c = const.tile([P, NBLK], F16, name="sig16c")
        mu16c = const.tile([P, NBLK], F16, name="mu16c")

        for kt in range(KT):
            nc.sync.dma_start(xts[kt][:], xT_d[kt * P:(kt + 1) * P, :])
        nc.sync.dma_start(wqk_sb[:], wqk_d[:])
        nc.sync.dma_start(wv_sb[:], wv_d[:])
        nc.sync.dma_start(wout_sb[:], wout_d[:])
        nc.sync.dma_start(ncsqk[:], ncsqk_d[:])
        nc.sync.dma_start(qkb[:], qkb_d[:])
        nc.sync.dma_start(ncsv[:], ncsv_d[:])
        nc.sync.dma_start(vb1[:], vb1_d[:])
        nc.sync.dma_start(maskc[:], mask_d[:])
        nc.sync.dma_start(ind2[:], ind2_d[:])
        nc.sync.dma_start(ind2f[:], ind2f_d[:])
        nc.sync.dma_start(id128[:], id128_d[:])
        nc.vector.memset(ones1[:], 1.0)
        nc.vector.memset(epsA[:], LN_EPS)
        nc.vector.memset(epsB[:], L2_EPS)
        nc.vector.memset(ones2[:], 0.0)
        nc.vector.memset(ones2[0:64, 0:1], 1.0)
        nc.vector.memset(ones2[64:128, 1:2], 1.0)

        # pp pool opens before phase 1 so j=0's q/k strip matmuls can be
        # hoisted under the phase-1 square chain (2 + 4 psum banks coexist).
        pp = ctx.enter_context(tc.tile_pool(name="pp", bufs=2, space="PSUM"))

        def emit_projqk_mms_hoisted():
            tiles = []
            for c0 in (0, 128):
                pq = pp.tile([P, 512], F32, name="pp")
                for kt in range(KT):
                    nc.tensor.matmul(pq[:], lhsT=wqk_sb[:, kt, c0:c0 + 128],
                                     rhs=xts[kt][:, 0:512],
                                     start=(kt == 0), stop=False)
                tiles.append(pq)
            return tiles

        # ---- Phase 1: LN stats on raw x, token-partition layout ----
        # s1/s2 column sums via free-size-1 PE matmuls accumulating over
        # strips; squares split DVE/ACT so neither serializes.
        # NOTE: accumulation groups targeting different regions of one psum
        # tile must NOT interleave (measured: interleaving corrupts the
        # accumulation). Keep each [128,1] column's strip-loop contiguous.
        with tc.tile_pool(name="p1sb", bufs=1) as p1sb, \
             tc.tile_pool(name="p1sq", bufs=1) as p1sq, \
             tc.tile_pool(name="p1ps", bufs=1, space="PSUM") as p1ps:
            ps1t = p1ps.tile([P, NBLK], F32, name="ps1t")
            ps2t = p1ps.tile([P, NBLK], F32, name="ps2t")
            sqs = []
            for kt in range(KT):
                sq = p1sq.tile([P, N], F16, name=f"sqt{kt}")
                if kt % 4 == 3:
                    nc.scalar.activation(sq[:], xts[kt][:], AF.Square)
                else:
                    nc.vector.tensor_tensor(sq[:], xts[kt][:], xts[kt][:],
                                            OP.mult)
                sqs.append(sq)
            for nb in range(NBLK):
                bs = slice(nb * P, (nb + 1) * P)
                for kt in range(KT):
                    nc.tensor.matmul(ps1t[:, nb:nb + 1], lhsT=xts[kt][:, bs],
                                     rhs=ones1[:], start=(kt == 0),
                                     stop=(kt == KT - 1))
            for nb in range(NBLK):
                bs = slice(nb * P, (nb + 1) * P)
                for kt in range(KT):
                    nc.tensor.matmul(ps2t[:, nb:nb + 1], lhsT=sqs[kt][:, bs],
                                     rhs=ones1[:], start=(kt == 0),
                                     stop=(kt == KT - 1))

            qk0_tiles = emit_projqk_mms_hoisted()

            # mu = s1/D ; var = s2/D - mu^2 ; sig = sqrt(var+eps) via
            # exp(0.5 ln), rsig = exp(-0.5 ln)
            bsq = p1sb.tile([P, NBLK], F32, name="bsq")
            av = p1sb.tile([P, NBLK], F32, name="av")
            lnv = p1sb.tile([P, NBLK], F32, name="lnv")
            nc.vector.tensor_scalar(out=mu16c[:], in0=ps1t[:], scalar1=INV_D,
                                    scalar2=None, op0=OP.mult)
            nc.vector.tensor_tensor(bsq[:], mu16c[:], mu16c[:], OP.mult)
            nc.vector.tensor_scalar(out=av[:], in0=ps2t[:], scalar1=INV_D,
                                    scalar2=None, op0=OP.mult)
            nc.vector.tensor_tensor(av[:], av[:], bsq[:], OP.subtract)
            nc.scalar.activation(lnv[:], av[:], AF.Ln, bias=epsA[:])
            nc.scalar.activation(sig16c[:], lnv[:], AF.Exp, scale=0.5)
            nc.scalar.activation(rsigc[:], lnv[:], AF.Exp, scale=-0.5)

            # transpose mu/sig to [16,128] then pack-DMA to [1, 2048] rows
            pmut = p1ps.tile([NBLK, P], F16, name="pmut")
            psigt = p1ps.tile([NBLK, P], F16, name="psigt")
            nc.tensor.transpose(pmut[:], mu16c[:], id128[:])
            nc.tensor.transpose(psigt[:], sig16c[:], id128[:])
            mur = p1sb.tile([NBLK, P], F16, name="mur")
            sigr = p1sb.tile([NBLK, P], F16, name="sigr")
            nc.vector.tensor_copy(mur[:], pmut[:])
            nc.vector.tensor_copy(sigr[:], psigt[:])
            for nb in range(NBLK):
                nc.sync.dma_start(mu1[0:1, nb * P:(nb + 1) * P],
                                  mur[nb:nb + 1, :])
                nc.sync.dma_start(sig1[0:1, nb * P:(nb + 1) * P],
                                  sigr[nb:nb + 1, :])

        # ---- Phase 2: software-pipelined per-megablock wavefront ----
        ps2p = ctx.enter_context(tc.tile_pool(name="ps2", bufs=2, space="PSUM"))
        pop = ctx.enter_context(tc.tile_pool(name="po", bufs=1, space="PSUM"))
        sqp = ctx.enter_context(tc.tile_pool(name="sq", bufs=2))
        smallp = ctx.enter_context(tc.tile_pool(name="sm", bufs=2))
        epool = ctx.enter_context(tc.tile_pool(name="ep", bufs=8))
        ocp = ctx.enter_context(tc.tile_pool(name="oc", bufs=2))
        rdp = ctx.enter_context(tc.tile_pool(name="rd", bufs=2))
        bdp = ctx.enter_context(tc.tile_pool(name="bd", bufs=2))
        outp = ctx.enter_context(tc.tile_pool(name="out", bufs=3))

        def emit_projqk_mms(j):
            js = slice(j * 512, (j + 1) * 512)
            tiles = []
            for c0 in (0, 128):
                pq = pp.tile([P, 512], F32, name="pp")
                for kt in range(KT):
                    nc.tensor.matmul(pq[:], lhsT=wqk_sb[:, kt, c0:c0 + 128],
                                     rhs=xts[kt][:, js],
                                     start=(kt == 0), stop=False)
                tiles.append(pq)
            return tiles

        def emit_projqk_finish(j, tiles):
            js = slice(j * 512, (j + 1) * 512)
            for ti, T in enumerate((qT, kT)):
                c0 = ti * 128
                pq = tiles[ti]
                nc.tensor.matmul(pq[:], lhsT=ncsqk[:, c0:c0 + 128],
                                 rhs=mu1[:, js], start=False, stop=False)
                nc.tensor.matmul(pq[:], lhsT=qkb[:, c0:c0 + 128],
                                 rhs=sig1[:, js], start=False, stop=True)
                nc.vector.tensor_copy(T[:, js], pq[:])

        def emit_sq2(j):
            js = slice(j * 512, (j + 1) * 512)
            sq2 = sqp.tile([P, 1024], F16, name="sq2")
            nc.vector.tensor_tensor(sq2[:, 0:512], qT[:, js], qT[:, js],
                                    OP.mult)
            nc.vector.tensor_tensor(sq2[:, 512:1024], kT[:, js], kT[:, js],
                                    OP.mult)
            return sq2

        def emit_normsum(j, sq2):
            """per-head norm sums -> ACT ln/exp chain"""
            rqks = []
            for ti in range(2):
                pn = pp.tile([P, 512], F32, name="pp")
                nc.tensor.matmul(pn[0:2, :], lhsT=ones2[:],
                                 rhs=sq2[:, ti * 512:(ti + 1) * 512],
                                 start=True, stop=True)
                lnn = smallp.tile([2, 512], F32, name="lnn")
                nc.scalar.activation(lnn[:], pn[0:2, :], AF.Ln,
                                     bias=epsB[0:2, :])
                rq2 = smallp.tile([2, 512], F16, name="rq2")
                nc.scalar.activation(rq2[:], lnn[:], AF.Exp, scale=-0.5)
                rqks.append(rq2)
            return rqks

        def emit_pv_mms(j):
            pvs = []
            for nb in range(4 * j, 4 * j + 4):
                bs = slice(nb * P, (nb + 1) * P)
                pv = pp.tile([P, 512], F32, name="pp")
                for kt in range(KT):
                    nc.tensor.matmul(pv[:, 0:130],
                                     lhsT=xts[kt][:, bs],
                                     rhs=wv_sb[:, kt, :],
                                     start=(kt == 0), stop=False)
                nc.tensor.matmul(pv[:, 0:130], lhsT=mu1[:, bs], rhs=ncsv[:],
                                 start=False, stop=False)
                nc.tensor.matmul(pv[:, 0:130], lhsT=sig1[:, bs], rhs=vb1[:],
                                 start=False, stop=True)
                pvs.append((nb, pv))
            return pvs

        def emit_vdrains(pvs):
            for nb, pv in pvs:
                with nc.allow_low_precision(reason="v values are O(1)"):
                    nc.vector.tensor_scalar(out=V[:, nb, :], in0=pv[:, 0:130],
                                            scalar1=rsigc[:, nb:nb + 1],
                                            scalar2=None, op0=OP.mult)

        def emit_normapply(j, rqks):
            js = slice(j * 512, (j + 1) * 512)
            for ti, T in enumerate((qT, kT)):
                pb = pp.tile([P, 512], F32, name="pp")
                nc.tensor.matmul(pb[:], lhsT=ind2[:], rhs=rqks[ti][:],
                                 start=True, stop=True)
                with nc.allow_low_precision(reason="normalized q/k are O(1)"):
                    nc.vector.tensor_tensor(T[:, js], T[:, js], pb[:], OP.mult)

        def emit_attn(j):
            """QK one step ahead of PV; exp+mask between; den recips at end.
            Returns (po tiles, rden2) for the deferred tail."""
            js = slice(j * 512, (j + 1) * 512)
            KB = 4 * j + 4
            rden2 = rdp.tile([2, 512], F32, name="rden2")
            pos = []
            for h in range(2):
                hs = slice(h * 64, (h + 1) * 64)
                po = pop.tile([65, 512], F32, name=f"po{h}")
                pending = None
                for kp in range(KB // 2):
                    kb0, kb1 = 2 * kp, 2 * kp + 1
                    s2t = ps2p.tile([P, 1024], F32, name="s2t")
                    nc.tensor.matmul(s2t[:, 0:512],
                                     lhsT=kT[hs, kb0 * P:(kb0 + 1) * P],
                                     rhs=qT[hs, js], start=True, stop=True)
                    nc.tensor.matmul(s2t[:, 512:1024],
                                     lhsT=kT[hs, kb1 * P:(kb1 + 1) * P],
                                     rhs=qT[hs, js], start=True, stop=True)
                    E2 = epool.tile([P, 1024], F16, name="E2")
                    nc.scalar.activation(E2[:], s2t[:], AF.Exp, scale=SCALE)
                    if kb0 >= 4 * j:
                        r0 = kb0 - 4 * j
                        nc.vector.tensor_tensor(
                            E2[:], E2[:],
                            maskc[:, r0 * 512:(r0 + 2) * 512], OP.mult)
                    if pending is not None:
                        pE, pk0, pk1 = pending
                        nc.tensor.matmul(po[:],
                                         lhsT=V[:, pk0, h * 65:(h + 1) * 65],
                                         rhs=pE[:, 0:512],
                                         start=(pk0 == 0), stop=False)
                        nc.tensor.matmul(po[:],
                                         lhsT=V[:, pk1, h * 65:(h + 1) * 65],
                                         rhs=pE[:, 512:1024],
                                         start=False, stop=False)
                    pending = (E2, kb0, kb1)
                pE, pk0, pk1 = pending
                nc.tensor.matmul(po[:], lhsT=V[:, pk0, h * 65:(h + 1) * 65],
                                 rhs=pE[:, 0:512], start=(pk0 == 0),
                                 stop=False)
                nc.tensor.matmul(po[:], lhsT=V[:, pk1, h * 65:(h + 1) * 65],
                                 rhs=pE[:, 512:1024], start=False, stop=True)
                pos.append(po)
                rdt = rdp.tile([65, 512], F32, name="rdt")
                nc.vector.reciprocal(rdt[64:65, :], po[64:65, :])
                nc.sync.dma_start(rden2[h:h + 1, :], rdt[64:65, :])
            return pos, rden2

        def emit_tail_bcast(pos, rden2):
            """den broadcast + divide-in-drain for a finished block."""
            pbd = pp.tile([P, 512], F32, name="pp")
            nc.tensor.matmul(pbd[:], lhsT=ind2f[:], rhs=rden2[:],
                             start=True, stop=True)
            bden = bdp.tile([P, 512], F32, name="bden")
            nc.scalar.copy(bden[:], pbd[:])
            ocat = ocp.tile([P, 512], F16, name="ocat")
            oc1 = ocp.tile([64, 512], F16, name="oc1")
            with nc.allow_low_precision(reason="attention out is O(1)"):
                nc.vector.tensor_tensor(ocat[0:64, :], pos[0][0:64, :],
                                        bden[0:64, :], OP.mult)
                nc.vector.tensor_tensor(oc1[:], pos[1][0:64, :],
                                        bden[64:128, :], OP.mult)
            nc.sync.dma_start(ocat[64:128, :], oc1[:])
            return ocat

        def emit_tail_outproj(j, ocat):
            for qb in range(4):
                row = (j * 4 + qb) * P
                yt = outp.tile([P, DIM], F16, name="yt")
                for ns in range(2):
                    py = pp.tile([P, 512], F32, name="pp")
                    nc.tensor.matmul(py[:],
                                     lhsT=ocat[:, qb * P:(qb + 1) * P],
                                     rhs=wout_sb[:, ns * 512:(ns + 1) * 512],
                                     start=True, stop=True)
                    if ns == 0:
                        nc.vector.tensor_copy(yt[:, 0:512], py[:])
                    else:
                        nc.scalar.copy(yt[:, 512:1024], py[:])
                nc.sync.dma_start(y_d[row:row + P, :], yt[:])

        tail = None
        qk_tiles = qk0_tiles
        for j in range(QM):
            if qk_tiles is None:
                qk_tiles = emit_projqk_mms(j)
            emit_projqk_finish(j, qk_tiles)
            qk_tiles = None
            ocat = None
            if tail is not None:
                tj, tpos, trden = tail
                ocat = emit_tail_bcast(tpos, trden)
            sq2 = emit_sq2(j)
            rqks = emit_normsum(j, sq2)
            pvs = emit_pv_mms(j)
            emit_vdrains(pvs)
            if ocat is not None:
                emit_tail_outproj(tj, ocat)
            emit_normapply(j, rqks)
            tail = (j,) + emit_attn(j)
        tj, tpos, trden = tail
        ocat = emit_tail_bcast(tpos, trden)
        emit_tail_outproj(tj, ocat)

    nc.compile()
    return nc


def make_in_maps(x, ln_w, ln_b, w_qkv, w_out):
    x = np.asarray(x, np.float32)
    ln_w = np.asarray(ln_w, np.float32)
    ln_b = np.asarray(ln_b, np.float32)
    w_qkv = np.asarray(w_qkv, np.float32)
    w_out = np.asarray(w_out, np.float32)

    maskc = np.zeros((P, 4 * 512), np.float16)
    for r in range(4):
        kk = np.arange(P)[:, None] + 128 * r
        qq = np.arange(512)[None, :]
        maskc[:, r * 512:(r + 1) * 512] = (kk <= qq).astype(np.float16)
    ind2 = np.zeros((2, P), np.float16)
    ind2[0, 0:64] = 1.0
    ind2[1, 64:128] = 1.0
    ind2f = ind2.astype(np.float32)
    id128 = np.eye(P, dtype=np.float16)

    in_maps = []
    for core in range(N_CORES):
        b, h0 = core // 4, (core % 4) * 2
        cs = [slice(base + h0 * D, base + (h0 + 2) * D)
              for base in (0, 512, 1024)]
        Wq = (w_qkv[:, cs[0]] * ln_w[:, None]).astype(np.float32)
        Wk = (w_qkv[:, cs[1]] * ln_w[:, None]).astype(np.float32)
        Wv = (w_qkv[:, cs[2]] * ln_w[:, None]).astype(np.float32)
        qb_ = ln_b @ w_qkv[:, cs[0]]
        kb_ = ln_b @ w_qkv[:, cs[1]]
        vb_ = ln_b @ w_qkv[:, cs[2]]

        wqk = np.concatenate([Wq, Wk], axis=1).astype(np.float16)
        wqk = np.ascontiguousarray(
            wqk.reshape(KT, P, 256).transpose(1, 0, 2))
        wv = np.zeros((DIM, 130), np.float32)
        wv[:, 0:64] = Wv[:, 0:64]
        wv[:, 65:129] = Wv[:, 64:128]
        wv = np.ascontiguousarray(
            wv.astype(np.float16).reshape(KT, P, 130).transpose(1, 0, 2))

        ncsqk = -np.concatenate([Wq.sum(0), Wk.sum(0)]).astype(np.float16)
        qkb = np.concatenate([qb_, kb_]).astype(np.float16)
        ncsv = np.zeros((130,), np.float32)
        ncsv[0:64] = -Wv.sum(0)[0:64]
        ncsv[65:129] = -Wv.sum(0)[64:128]
        vb1 = np.zeros((130,), np.float32)
        vb1[0:64] = vb_[0:64]
        vb1[65:129] = vb_[64:128]
        vb1[64] = 1.0
        vb1[129] = 1.0

        in_maps.append({
            "xT": np.ascontiguousarray(x[b].T.astype(np.float16)),
            "wqk": wqk,
            "wv": wv,
            "wout": np.ascontiguousarray(
                w_out[h0 * D:(h0 + 2) * D]).astype(np.float16),
            "ncsqk": ncsqk[None, :].astype(np.float16),
            "qkb": qkb[None, :].astype(np.float16),
            "ncsv": ncsv[None, :].astype(np.float16),
            "vb1": vb1[None, :].astype(np.float16),
            "maskc": maskc,
            "ind2": ind2,
            "ind2f": ind2f,
            "id128": id128,
        })
    return in_maps


def kernel(x, ln_w, ln_b, w_qkv, w_out):
    if "nc" not in _CACHE:
        _CACHE["nc"] = build_nc()
    nc = _CACHE["nc"]
    in_maps = make_in_maps(x, ln_w, ln_b, w_qkv, w_out)
    res = bass_utils.run_bass_kernel_spmd(nc, in_maps,
                                          core_ids=list(range(N_CORES)))
    y = np.zeros((B, N, DIM), np.float32)
    for core in range(N_CORES):
        y[core // 4] += res.results[core]["y"].astype(np.float32)
    return y
